# revision 4
# baseline (speedup 1.0000x reference)
"""Trainium2 Bass kernel for nn_BaseAggregator_31439160607279 (v3).

Math (reference):
  af (a,c,f,t), imf (v,c,h,w), split c into k=2 heads of 256 ch.
  sims[a,v,k,hw,t] = sum_c af*imf ; + cls[a,v,k] ; relu ; max over hw ;
  masked mean over t (mask m[a,t] in {0,1}, den = f*sum_t m) ; sum over k.

Strategy v3g:
  - 2D shard: RS=2 row-shards x VS=4 image-shards (VL=8 images/core).
  - Mask-active (a,t) rows packed; MTloc 128-row tiles per core.
  - Per (mt,k): psum tile A [128,1536] (blocks 0-5, 3 N=512 DR matmuls,
    drained by ONE Act copy) + psum tile B [128,512] (blocks 6-7 + junk,
    1 matmul, drained by ONE DVE reduce_max). Independent lifetimes ->
    PE never waits on the slower drain path.
  - DVE folds: L1+L2 per mt (cm -> m2 -> m49), L3/L4/reduce-13 per 2mt.
  - cls-add on gpsimd (mid-loop) / DVE (tail); relu on Act.
  - Input DMA on 3 queues (sync: imf k0, scalar: imf k1, gpsimd: afp),
    chunked tiles so mt0 starts as soon as its chunks land.
"""

import math
from contextlib import ExitStack

import numpy as np
import ml_dtypes

import concourse.bacc as bacc
import concourse.mybir as mybir
import concourse.tile as tile
from concourse.bass_utils import run_bass_kernel_spmd

A, V, C, F, T, H, W = 32, 32, 512, 1, 200, 14, 14
K = 2
KP = 128
HW = H * W               # 196
NCORES = 8

RS, VS = 2, 4
VL = V // VS             # 8 images per core
NB = VL                  # 8 blocks (images) per (mt,k)
P1B = 2                  # blocks 6,7: DVE direct reduce (tile B)
P3B = NB - P1B           # blocks 0-5: Act copy + DVE folds (tile A)
CSPAN = P3B * HW         # 1176

TRACE = False
LAST_RESULTS = None
_kernel_cache = {}

f32 = mybir.dt.float32
f16 = mybir.dt.float16
f8 = mybir.dt.float8e4
X = mybir.AxisListType.X
DR = mybir.MatmulPerfMode.DoubleRow
MX = mybir.AluOpType.max
AD = mybir.AluOpType.add
RELU = mybir.ActivationFunctionType.Relu


def _build(MTloc: int):
    nc = bacc.Bacc("TRN2", target_bir_lowering=False, debug=False)

    afp_d = nc.dram_tensor("afp", (K, KP, MTloc * 256), f8, kind="ExternalInput")
    imf_d = nc.dram_tensor("imf", (K, KP, 4096), f8, kind="ExternalInput")
    clsb_d = nc.dram_tensor("clsb", (KP, MTloc * 16), f16, kind="ExternalInput")
    mkd_d = nc.dram_tensor("mkd", (KP, MTloc * A), f16, kind="ExternalInput")
    outk_d = nc.dram_tensor("outk", (A, 16), f32, kind="ExternalOutput")

    with tile.TileContext(nc) as tc, ExitStack() as ctx:
        cst = ctx.enter_context(tc.tile_pool(name="cst", bufs=1))
        ps = ctx.enter_context(tc.tile_pool(name="ps", bufs=2, space="PSUM"))
        psb = ctx.enter_context(tc.tile_pool(name="psb", bufs=2, space="PSUM"))
        cmp_ = ctx.enter_context(tc.tile_pool(name="cmp", bufs=5))
        mp = ctx.enter_context(tc.tile_pool(name="mp", bufs=5))
        fld = ctx.enter_context(tc.tile_pool(name="fld", bufs=8))

        # afp chunk c covers mts [AFP_CH[c], AFP_CH[c+1])
        AFP_CH = [0, 1, 2, 6, 10, MTloc]
        while AFP_CH[-2] >= MTloc:
            AFP_CH.pop(-2)
        afp_sb = [[cst.tile([KP, (AFP_CH[c + 1] - AFP_CH[c]) * 256], f8,
                            tag=f"afp{k}_{c}", name=f"afp{k}_{c}")
                   for c in range(len(AFP_CH) - 1)] for k in range(K)]
        imf_sb = [[cst.tile([KP, 1024], f8, tag=f"imf{k}_{j}",
                            name=f"imf{k}_{j}") for j in range(4)]
                  for k in range(K)]
        clsb_sb = cst.tile([KP, MTloc * 16], f16, tag="clsb", name="clsb")
        mkd_sb = cst.tile([KP, MTloc * A], f16, tag="mkd", name="mkd")
        smraw = cst.tile([KP, MTloc * 16], f16, tag="smraw", name="smraw")
        sm3 = cst.tile([KP, MTloc * 16], f16, tag="sm3", name="sm3")

        # ---- input DMA on 3 queues ----
        for j in range(4):
            nc.sync.dma_start(out=imf_sb[0][j][:],
                              in_=imf_d.ap()[0][:, j * 1024:(j + 1) * 1024])
            nc.scalar.dma_start(out=imf_sb[1][j][:],
                                in_=imf_d.ap()[1][:, j * 1024:(j + 1) * 1024])
            if j == 0:
                for c in range(2):
                    for k in range(K):
                        nc.gpsimd.dma_start(
                            out=afp_sb[k][c][:],
                            in_=afp_d.ap()[k][:, AFP_CH[c] * 256:
                                               AFP_CH[c + 1] * 256])
        nc.scalar.dma_start(out=clsb_sb[:], in_=clsb_d.ap())
        nc.scalar.dma_start(out=mkd_sb[:], in_=mkd_d.ap())
        for c in range(2, len(AFP_CH) - 1):
            for k in range(K):
                nc.gpsimd.dma_start(
                    out=afp_sb[k][c][:],
                    in_=afp_d.ap()[k][:, AFP_CH[c] * 256:AFP_CH[c + 1] * 256])

        # ---- PE warm-up during DMA ----
        warm = cst.tile([KP, 512], f16, tag="warm", name="warm")
        nc.vector.memset(warm[:], 0.0)
        zero32 = cst.tile([KP, 32], f16, tag="zero32", name="zero32")
        nc.gpsimd.memset(zero32[:], 0.0)
        for _ in range(5):
            pw = ps.tile([128, 1536], f32, tag="ps", name="pw")
            nc.tensor.matmul(pw[:, 0:512], lhsT=warm[:, 0:128], rhs=warm[:],
                             start=True, stop=True)

        # ---- main loop ----
        def afp_chunk(c):
            for ci in range(len(AFP_CH) - 1):
                if AFP_CH[ci] <= c < AFP_CH[ci + 1]:
                    return ci, c - AFP_CH[ci]
            raise AssertionError

        def emit_tile(mt, k):
            ci, off = afp_chunk(mt)
            stat = afp_sb[k][ci][:, off * 256:(off + 1) * 256].rearrange(
                "p (i m) -> p i m", i=2)
            pa = ps.tile([128, 1536], f32, tag="ps", name=f"a{mt}_{k}")
            for j in range(3):
                rv = imf_sb[k][j][:].rearrange("p (i n) -> p i n", i=2)
                nc.tensor.matmul(pa[:, j * 512:(j + 1) * 512], lhsT=stat,
                                 rhs=rv, start=True, stop=True, perf_mode=DR)
            pb = psb.tile([128, 512], f32, tag="psb", name=f"b{mt}_{k}")
            rv3 = imf_sb[k][3][:].rearrange("p (i n) -> p i n", i=2)
            nc.tensor.matmul(pb[:], lhsT=stat, rhs=rv3, start=True, stop=True,
                             perf_mode=DR)
            return pa, pb

        def fold_tail(m0, nmt, on_gp):
            # L3/L4 + reduce-13 from m49, then cls-add + relu
            nb = nmt * 2 * P3B   # 12 or 24 blocks
            m49v = m49[:, 0:nb * 49].rearrange("p (b c) -> p b c", b=nb)
            m25 = fld.tile([KP, 24 * 25], f16, tag="m25", name=f"m25_{m0}")
            m25v = m25[:, 0:nb * 25].rearrange("p (b c) -> p b c", b=nb)
            nc.vector.tensor_tensor(m25v, m49v[:, :, 0:25],
                                    m49v[:, :, 24:49], MX)
            m13 = fld.tile([KP, 24 * 13], f16, tag="m13", name=f"m13_{m0}")
            m13v = m13[:, 0:nb * 13].rearrange("p (b c) -> p b c", b=nb)
            nc.vector.tensor_tensor(m13v, m25v[:, :, 0:13],
                                    m25v[:, :, 12:25], MX)
            ov = smraw[:, m0 * 16:(m0 + nmt) * 16].rearrange(
                "p (m t b) -> p m t b", m=nmt, t=2)[:, :, :, 0:P3B]
            nc.vector.reduce_max(ov, m13v, axis=X)
            lo, hi = m0 * 16, (m0 + nmt) * 16
            eng2 = nc.gpsimd if on_gp else nc.vector
            eng2.tensor_tensor(sm3[:, lo:hi], smraw[:, lo:hi],
                               clsb_sb[:, lo:hi], AD)
            nc.vector.tensor_scalar_max(sm3[:, lo:hi], sm3[:, lo:hi], 0.0)

        m2 = None
        m49 = None
        for mt in range(MTloc):
            if mt % 2 == 0:
                m2 = mp.tile([KP, 24 * 98], f16, tag="m2", name=f"m2_{mt}")
                m49 = fld.tile([KP, 24 * 49], f16, tag="m49", name=f"m49_{mt}")
            cm = cmp_.tile([KP, 2 * CSPAN], f16, tag="cm", name=f"cm_{mt}")
            for kt in range(K):
                pa, pb = emit_tile(mt, kt)
                nc.scalar.copy(cm[:, kt * CSPAN:(kt + 1) * CSPAN],
                               pa[:, 0:CSPAN])
                base = mt * 16 + kt * 8
                nc.vector.reduce_max(
                    smraw[:, base + P3B:base + NB],
                    pb[:, 0:P1B * HW].rearrange("p (b c) -> p b c", b=P1B),
                    axis=X)
            # L1 + L2 for this mt's 12 copied blocks
            cv = cm[:].rearrange("p (t b h c) -> p t b h c", t=2, b=P3B, h=2)
            mo = (mt % 2) * 12 * 98
            mv = m2[:, mo:mo + 12 * 98].rearrange("p (t b c) -> p t b c",
                                                  t=2, b=P3B)
            nc.vector.tensor_tensor(mv, cv[:, :, :, 0, :], cv[:, :, :, 1, :],
                                    MX)
            mo49 = (mt % 2) * 12 * 49
            m2vm = m2[:, mo:mo + 12 * 98].rearrange("p (b c) -> p b c", b=12)
            m49vm = m49[:, mo49:mo49 + 12 * 49].rearrange(
                "p (b c) -> p b c", b=12)
            nc.vector.tensor_tensor(m49vm, m2vm[:, :, 0:49],
                                    m2vm[:, :, 49:98], MX)
            if mt % 2 == 1:
                fold_tail(mt - 1, 2, on_gp=(mt < MTloc - 2))
            elif mt == MTloc - 1:
                fold_tail(mt, 1, on_gp=False)

        # ---- tail: masked t-sum accumulation + out ----
        pnum = ps.tile([128, 1536], f32, tag="ps", name="pnum")
        for mt in range(MTloc):
            nc.tensor.matmul(pnum[0:A, 0:16],
                             lhsT=mkd_sb[:, mt * A:(mt + 1) * A],
                             rhs=sm3[:, mt * 16:(mt + 1) * 16],
                             start=(mt == 0), stop=(mt == MTloc - 1))
        outk_sb = cst.tile([A, 16], f32, tag="outk", name="outk")
        nc.vector.tensor_copy(outk_sb[:], pnum[0:A, 0:16])
        nc.sync.dma_start(out=outk_d.ap(), in_=outk_sb[:])

    nc.compile()
    return nc


def prepare_inputs(audio_feats, image_feats, audio_cls, image_cls, audio_mask):
    af5 = np.ascontiguousarray(audio_feats, np.float32).reshape(A, K, 2, KP, T)
    imf5 = np.ascontiguousarray(image_feats, np.float32).reshape(V, K, 2, KP, HW)
    maskb = np.asarray(audio_mask) != 0
    rows_a, rows_t = np.nonzero(maskb)
    L = len(rows_a)
    MTtot = max(1, math.ceil(L / 128))
    MTloc = max(1, math.ceil(MTtot / RS))
    cap = RS * MTloc * 128

    af_rows = np.zeros((cap, K, 2, KP), np.float32)
    af_rows[:L] = af5[rows_a, :, :, :, rows_t]
    a_of_row = np.full(cap, -1, np.int64)
    a_of_row[:L] = rows_a

    cls_full = np.einsum(
        "akc,vkc->avk",
        np.asarray(audio_cls, np.float32).reshape(A, K, C // K),
        np.asarray(image_cls, np.float32).reshape(V, K, C // K),
    ).astype(np.float32)
    rden = 1.0 / (F * maskb.sum(1).astype(np.float32))

    # psum stream: blocks 0-5 at [0:1176] (tile A), blocks 6,7 at
    # [1536:1928] (tile B); flat col = j*1024 + i*512 + (col % 512)
    imf8_all = []
    for vs in range(VS):
        stream = np.zeros((K, 2, KP, 2048), np.float32)  # [k, i, p, psumcol]
        for b in range(VL):
            v = vs * VL + b
            base = b * HW if b < P3B else 1536 + (b - P3B) * HW
            stream[:, :, :, base:base + HW] = imf5[v]
        flat = np.zeros((K, KP, 4096), np.float32)
        for j in range(4):
            for i in range(2):
                flat[:, :, j * 1024 + i * 512:j * 1024 + (i + 1) * 512] = \
                    stream[:, i, :, j * 512:(j + 1) * 512]
        imf8_all.append(flat.astype(ml_dtypes.float8_e4m3))

    in_maps = []
    for rs in range(RS):
        sl = slice(rs * MTloc * 128, (rs + 1) * MTloc * 128)
        chunk = af_rows[sl]
        a_chunk = a_of_row[sl]
        afp = np.ascontiguousarray(
            chunk.reshape(MTloc, 128, K, 2, KP).transpose(2, 4, 0, 3, 1)
            .reshape(K, KP, MTloc * 256)).astype(ml_dtypes.float8_e4m3)

        mkd = np.zeros((MTloc, 128, A), np.float32)
        rr = np.arange(MTloc * 128)
        valid = a_chunk >= 0
        mkd[rr[valid] // 128, rr[valid] % 128, a_chunk[valid]] = \
            rden[a_chunk[valid]]
        mkd = np.ascontiguousarray(
            mkd.transpose(1, 0, 2).reshape(128, MTloc * A)).astype(np.float16)

        for vs in range(VS):
            # slot s = k*8 + b -> (v = vs*8+b, head k)
            clsb = np.zeros((MTloc * 128, K, VL), np.float32)
            for k in range(K):
                cv = cls_full[:, vs * VL:(vs + 1) * VL, k]  # [A, VL]
                clsb[valid, k, :] = cv[a_chunk[valid]]
            clsb = np.ascontiguousarray(
                clsb.reshape(MTloc, 128, 16).transpose(1, 0, 2)
                .reshape(128, MTloc * 16)).astype(np.float16)
            in_maps.append({
                "afp": afp,
                "imf": imf8_all[vs],
                "clsb": clsb,
                "mkd": mkd,
            })
    return MTloc, in_maps


def get_program(MTloc: int):
    if MTloc not in _kernel_cache:
        _kernel_cache[MTloc] = _build(MTloc)
    return _kernel_cache[MTloc]


def kernel(audio_feats, image_feats, audio_cls, image_cls, audio_mask, agg_heads):
    global LAST_RESULTS
    MTloc, in_maps = prepare_inputs(
        audio_feats, image_feats, audio_cls, image_cls, audio_mask
    )
    nc = get_program(MTloc)
    res = run_bass_kernel_spmd(nc, in_maps, list(range(NCORES)), trace=TRACE)
    LAST_RESULTS = res
    agg = bool(np.asarray(agg_heads))
    outk = np.zeros((A, V, K), np.float32)
    for rs in range(RS):
        for vs in range(VS):
            o = np.asarray(res.results[rs * VS + vs]["outk"], np.float32)
            o = o.reshape(A, K, VL)
            for k in range(K):
                outk[:, vs * VL:(vs + 1) * VL, k] += o[:, k, :]
    if agg:
        return outk.sum(2).astype(np.float32)
    return outk.astype(np.float32)
